# revision 1
# baseline (speedup 1.0000x reference)
"""Multi-head self-attention (B=4, S=2048, D=1024, H=16) on 8 TRN2 NeuronCores.

Sharding: core i = (batch b = i//2, head-group g = i%2). Each core computes,
for its batch and its 8 heads: QKV projection, attention, and a partial
output projection over its 512 attention features. Host sums the two
partials per batch (Megatron-style tensor parallel over heads x data
parallel over batch).

Per-core dataflow (everything in the transposed orientation so the softmax
denominator can be computed on the tensor engine):
  V[t,e]            = x^T-stationary matmuls over Wv^T, stored with a ones
                      column per head (V_aug [t, 65] per head)
  QT/KT[f,s]        = Wq/Wk-stationary matmuls over x^T. Q is stored twice,
                      zero-padded per head half (qta: head-A rows live /
                      B rows zero, qtb: vice versa) so per-head scores run
                      as full K=128 matmuls without tile_position (which
                      faults on TRN2 for 4-byte dtypes).
  S^T[t,s]          = K^T-stationary matmuls against qta/qtb
  P^T[t,s]          = exp(S^T / 8)  (ScalarE, PSUM->SBUF; no max-sub:
                      scores are ~N(0,16) pre-scale so exp is safe in fp32)
  O^T_aug[e,s]      = V_aug-stationary matmuls over P^T (M=65); row 64
                      accumulates the softmax denominators for free
  On = O^T * recip(sums);  out^T[dout,s] = Wout^T-stationary over On.

Matmuls run in float32r (fp32 storage, ~bf16 rate on the PE array,
~1.5e-4 matmul rel err vs 2.3e-3 for bf16).
"""
import os
import sys
import types

import numpy as np

# ---------------------------------------------------------------------------
# environment bootstrap (self-contained: no problem-dir imports)
# ---------------------------------------------------------------------------


def _install_ntff_hook():
    """run_bass_kernel_spmd(trace=True) under axon needs antenv.axon_hooks,
    which the agent image's antenv stub lacks. Recreate it."""
    if "antenv.axon_hooks" in sys.modules:
        return
    try:
        import antenv
        from trn_agent_boot.trn_boot import _ntff_profile_via_ctypes
    except Exception:
        return
    so_path = "/opt/axon/libaxon_pjrt.so"
    if not os.path.exists(so_path):
        return
    mod = types.ModuleType("antenv.axon_hooks")
    _hook = [_ntff_profile_via_ctypes(so_path)]
    mod.get_axon_ntff_profile_hook = lambda: _hook[0]

    def _set(h):
        _hook[0] = h

    mod.set_axon_ntff_profile_hook = _set
    sys.modules["antenv.axon_hooks"] = mod
    antenv.axon_hooks = mod


_install_ntff_hook()

import concourse.bacc as bacc
import concourse.tile as tile
from concourse import mybir
from concourse.bass_utils import run_bass_kernel_spmd
from contextlib import ExitStack

# ---------------------------------------------------------------------------
# problem constants (hardcoded per contract)
# ---------------------------------------------------------------------------
B, S, D = 4, 2048, 1024
H, HD = 16, 64
HPG = 8            # heads per core (group)
E = HPG * HD       # 512 attention features per core
P = 128
SC = 512           # s-chunk
NS = S // SC       # 4 s-chunks
NT = S // P        # 16 t-chunks
ND = D // P        # 8 d-chunks
NF = E // P        # 4 f-chunks per Q (or K) = head-pairs
HD1 = HD + 1       # V_aug columns per head (V + ones)
SCALE = 1.0 / np.sqrt(np.float32(HD))

F32 = mybir.dt.float32
F32R = mybir.dt.float32r
EXP = mybir.ActivationFunctionType.Exp

_NC_CACHE = {}


def _build_nc():
    nc = bacc.Bacc("TRN2", target_bir_lowering=False)

    xT = nc.dram_tensor("xT", [D, S], F32R, kind="ExternalInput")
    wqT = nc.dram_tensor("wqT", [D, E], F32R, kind="ExternalInput")
    wkT = nc.dram_tensor("wkT", [D, E], F32R, kind="ExternalInput")
    wvT = nc.dram_tensor("wvT", [D, E], F32R, kind="ExternalInput")
    woT = nc.dram_tensor("woT", [E, D], F32R, kind="ExternalInput")
    bq = nc.dram_tensor("bq", [E, 1], F32, kind="ExternalInput")
    bk = nc.dram_tensor("bk", [E, 1], F32, kind="ExternalInput")
    bv = nc.dram_tensor("bv", [1, E], F32, kind="ExternalInput")
    bo = nc.dram_tensor("bo", [D, 1], F32, kind="ExternalInput")
    outT = nc.dram_tensor("outT", [D, S], F32, kind="ExternalOutput")

    with tile.TileContext(nc) as tc, ExitStack() as glob:
        const = glob.enter_context(tc.tile_pool(name="const", bufs=1))
        bv_bc = const.tile([P, E], F32, name="bv_bc")
        nc.sync.dma_start(bv_bc[:], bv[0:1, :].to_broadcast((P, E)))

        resid = glob.enter_context(tc.tile_pool(name="resid", bufs=1))
        # qta: head-A rows (0-63) live, rows 64-127 zero; qtb: opposite
        qta = [resid.tile([P, S], F32R, name=f"qta{f}") for f in range(NF)]
        qtb = [resid.tile([P, S], F32R, name=f"qtb{f}") for f in range(NF)]
        kt = [resid.tile([P, S], F32R, name=f"kt{f}") for f in range(NF)]
        vt = [resid.tile([P, HPG * HD1], F32R, name=f"vt{t}") for t in range(NT)]
        for f in range(NF):
            nc.vector.memset(qta[f][HD:P, :].bitcast(F32), 0.0)
            nc.vector.memset(qtb[f][0:HD, :].bitcast(F32), 0.0)

        # ---------------- phase 1a: V projection ------------------------
        with ExitStack() as c1:
            wvp = c1.enter_context(tc.tile_pool(name="wvp", bufs=1))
            wv = [wvp.tile([P, E], F32R, name=f"wv{d}") for d in range(ND)]
            for d in range(ND):
                nc.sync.dma_start(wv[d][:], wvT[d * P:(d + 1) * P, :])
            xpool = c1.enter_context(tc.tile_pool(name="xv", bufs=2))
            psv = c1.enter_context(tc.tile_pool(name="psv", bufs=4, space="PSUM"))
            for s in range(NS):
                sl = slice(s * SC, (s + 1) * SC)
                xts = [xpool.tile([P, SC], F32R, name="xts", tag=f"x{d}")
                       for d in range(ND)]
                for d in range(ND):
                    nc.sync.dma_start(xts[d][:], xT[d * P:(d + 1) * P, sl])
                for i in range(NS):
                    t = s * NS + i
                    ps = psv.tile([P, E], F32, name="psvt", tag="psv")
                    for d in range(ND):
                        nc.tensor.matmul(
                            ps[:], xts[d][:, i * P:(i + 1) * P], wv[d][:],
                            start=(d == 0), stop=(d == ND - 1))
                    vdst = vt[t][:].rearrange("p (h c) -> p h c", c=HD1)
                    nc.vector.tensor_add(
                        vdst[:, :, 0:HD],
                        ps[:].rearrange("p (h c) -> p h c", c=HD),
                        bv_bc[:].rearrange("p (h c) -> p h c", c=HD))
                    nc.vector.memset(vdst[:, :, HD:HD1].bitcast(F32), 1.0)

        # ---------------- phase 1b: Q/K projections ---------------------
        with ExitStack() as c1:
            wpool = c1.enter_context(tc.tile_pool(name="w", bufs=1))
            wq = [wpool.tile([P, E], F32R, name=f"wq{d}") for d in range(ND)]
            wk = [wpool.tile([P, E], F32R, name=f"wk{d}") for d in range(ND)]
            for d in range(ND):
                nc.sync.dma_start(wq[d][:], wqT[d * P:(d + 1) * P, :])
                nc.sync.dma_start(wk[d][:], wkT[d * P:(d + 1) * P, :])
            bqt = [wpool.tile([P, 1], F32, name=f"bqt{f}") for f in range(NF)]
            bkt = [wpool.tile([P, 1], F32, name=f"bkt{f}") for f in range(NF)]
            for f in range(NF):
                nc.sync.dma_start(bqt[f][:], bq[f * P:(f + 1) * P, :])
                nc.sync.dma_start(bkt[f][:], bk[f * P:(f + 1) * P, :])

            xpool = c1.enter_context(tc.tile_pool(name="xq", bufs=1))
            psq = c1.enter_context(tc.tile_pool(name="psq", bufs=4, space="PSUM"))

            for s in range(NS):
                sl = slice(s * SC, (s + 1) * SC)
                xts = [xpool.tile([P, SC], F32R, name="xts2", tag=f"x2{d}")
                       for d in range(ND)]
                for d in range(ND):
                    nc.sync.dma_start(xts[d][:], xT[d * P:(d + 1) * P, sl])
                for f in range(NF):
                    ps = psq.tile([P, SC], F32, name="psqt", tag="psq")
                    for d in range(ND):
                        nc.tensor.matmul(
                            ps[:], wq[d][:, f * P:(f + 1) * P], xts[d][:],
                            start=(d == 0), stop=(d == ND - 1))
                    nc.vector.tensor_scalar_add(
                        qta[f][0:HD, sl], ps[0:HD, :], bqt[f][0:HD, :])
                    nc.vector.tensor_scalar_add(
                        qtb[f][HD:P, sl], ps[HD:P, :], bqt[f][HD:P, :])
                for f in range(NF):
                    ps = psq.tile([P, SC], F32, name="pskt", tag="psq")
                    for d in range(ND):
                        nc.tensor.matmul(
                            ps[:], wk[d][:, f * P:(f + 1) * P], xts[d][:],
                            start=(d == 0), stop=(d == ND - 1))
                    nc.vector.tensor_scalar_add(kt[f][:, sl], ps[:], bkt[f][:])

        # ---------------- phase 2: attention + out-proj -----------------
        with ExitStack() as c2:
            wo_pool = c2.enter_context(tc.tile_pool(name="wo", bufs=1))
            wo = [wo_pool.tile([P, D], F32R, name=f"wo{e}") for e in range(NF)]
            for e in range(NF):
                nc.sync.dma_start(wo[e][:], woT[e * P:(e + 1) * P, :])
            bot = [wo_pool.tile([P, 1], F32, name=f"bot{i}") for i in range(ND)]
            for i in range(ND):
                nc.sync.dma_start(bot[i][:], bo[i * P:(i + 1) * P, :])

            dram_pool = c2.enter_context(tc.tile_pool(name="dramrs", bufs=2, space="DRAM"))
            pt_pool = c2.enter_context(tc.tile_pool(name="pt", bufs=4))
            on_pool = c2.enter_context(tc.tile_pool(name="on", bufs=2))
            rs_pool = c2.enter_context(tc.tile_pool(name="rs", bufs=2))
            rb_pool = c2.enter_context(tc.tile_pool(name="rb", bufs=2))
            ot_pool = c2.enter_context(tc.tile_pool(name="ot", bufs=3))
            ps_sc = c2.enter_context(tc.tile_pool(name="ps_sc", bufs=2, space="PSUM"))
            ps_o = c2.enter_context(tc.tile_pool(name="ps_o", bufs=1, space="PSUM"))
            ps_op = c2.enter_context(tc.tile_pool(name="ps_op", bufs=2, space="PSUM"))

            for s in range(NS):
                sl = slice(s * SC, (s + 1) * SC)
                on_tiles = [on_pool.tile([P, SC], F32R, name="on", tag=f"on{hp}")
                            for hp in range(NF)]
                for hp in range(NF):
                    hA, hB = 2 * hp, 2 * hp + 1
                    o_psA = ps_o.tile([P, SC], F32, name="opsA", tag="oA")
                    o_psB = ps_o.tile([P, SC], F32, name="opsB", tag="oB")
                    for t in range(NT):
                        tsl = slice(t * P, (t + 1) * P)
                        sc_ps = ps_sc.tile([P, 2 * SC], F32, name="scps", tag="sc")
                        # K=128 matmuls; zero rows in qta/qtb mask the other head
                        nc.tensor.matmul(
                            sc_ps[:, 0:SC], kt[hp][:, tsl], qta[hp][:, sl],
                            start=True, stop=True)
                        nc.tensor.matmul(
                            sc_ps[:, SC:2 * SC], kt[hp][:, tsl], qtb[hp][:, sl],
                            start=True, stop=True)
                        pt = pt_pool.tile([P, 2 * SC], F32R, name="ptile", tag="pt")
                        nc.scalar.activation(pt[:], sc_ps[:], EXP, scale=float(SCALE))
                        # PV with ones column: out rows 0-63 = O^T, row 64 = sums
                        nc.tensor.matmul(
                            o_psA[0:HD1, :],
                            vt[t][:, hA * HD1:(hA + 1) * HD1],
                            pt[:, 0:SC],
                            start=(t == 0), stop=(t == NT - 1))
                        nc.tensor.matmul(
                            o_psB[0:HD1, :],
                            vt[t][:, hB * HD1:(hB + 1) * HD1],
                            pt[:, SC:2 * SC],
                            start=(t == 0), stop=(t == NT - 1))
                    # evict O_aug to SBUF quickly (frees PSUM for next head),
                    # then normalize off the critical path: broadcast the raw
                    # sums row via a DRAM bounce and divide on DVE.
                    ocA = rs_pool.tile([P, SC], F32, name="ocA", tag="ocA")
                    ocB = rs_pool.tile([P, SC], F32, name="ocB", tag="ocB")
                    nc.vector.tensor_copy(ocA[0:HD1, :], o_psA[0:HD1, :])
                    nc.vector.tensor_copy(ocB[0:HD1, :], o_psB[0:HD1, :])
                    # reciprocal of the two sums rows on all 128 DVE lanes:
                    # bounce each [1,512] row through DRAM, reload as [64,8]
                    # partition-spread, one reciprocal, bounce back.
                    rd = dram_pool.tile([2, SC], F32, name="rdtile", tag="rd")
                    nc.sync.dma_start(rd[0:1, :], ocA[HD:HD1, :])
                    nc.sync.dma_start(rd[1:2, :], ocB[HD:HD1, :])
                    rsp = rs_pool.tile([P, SC // HD], F32, name="rsp", tag="rsp")
                    nc.sync.dma_start(
                        rsp[0:HD, :],
                        rd[0:1, :].rearrange("a (p c) -> (a p) c", c=SC // HD))
                    nc.sync.dma_start(
                        rsp[HD:P, :],
                        rd[1:2, :].rearrange("a (p c) -> (a p) c", c=SC // HD))
                    nc.vector.reciprocal(rsp[:], rsp[:])
                    nc.sync.dma_start(
                        rd[0:1, :].rearrange("a (p c) -> (a p) c", c=SC // HD),
                        rsp[0:HD, :])
                    nc.sync.dma_start(
                        rd[1:2, :].rearrange("a (p c) -> (a p) c", c=SC // HD),
                        rsp[HD:P, :])
                    rb = rb_pool.tile([HD, SC], F32, name="rbtile", tag="rb")
                    rb2 = rb_pool.tile([HD, SC], F32, name="rb2tile", tag="rb2")
                    nc.sync.dma_start(rb[0:HD, :], rd[0:1, :].to_broadcast((HD, SC)))
                    nc.sync.dma_start(rb2[0:HD, :], rd[1:2, :].to_broadcast((HD, SC)))
                    # head A -> partitions 0-63 directly; head B -> via SBUF
                    # tmp then a DMA partition-shift to 64-127
                    nc.vector.tensor_mul(
                        on_tiles[hp][0:HD, :], ocA[0:HD, :], rb[0:HD, :])
                    tmpB = rb_pool.tile([HD, SC], F32R, name="tmpB", tag="tmpB")
                    nc.vector.tensor_mul(
                        tmpB[0:HD, :], ocB[0:HD, :], rb2[0:HD, :])
                    nc.sync.dma_start(on_tiles[hp][HD:P, :], tmpB[0:HD, :])
                # output projection for this s-chunk
                for dc in range(ND):
                    op_ps = ps_op.tile([P, SC], F32, name="opps", tag="op")
                    for e in range(NF):
                        nc.tensor.matmul(
                            op_ps[:], wo[e][:, dc * P:(dc + 1) * P], on_tiles[e][:],
                            start=(e == 0), stop=(e == NF - 1))
                    ot = ot_pool.tile([P, SC], F32, name="ottile", tag="ot")
                    nc.vector.tensor_scalar_add(ot[:], op_ps[:], bot[dc][:])
                    nc.sync.dma_start(outT[dc * P:(dc + 1) * P, sl], ot[:])

    nc.finalize()
    return nc


def _get_nc():
    if "nc" not in _NC_CACHE:
        _NC_CACHE["nc"] = _build_nc()
    return _NC_CACHE["nc"]


def _shard_inputs(x, w_qkv, b_qkv, w_out, b_out):
    """Build the 8 per-core input maps. Core i = (b = i//2, g = i%2)."""
    x = np.asarray(x, np.float32)
    w_qkv = np.asarray(w_qkv, np.float32)
    b_qkv = np.asarray(b_qkv, np.float32)
    w_out = np.asarray(w_out, np.float32)
    b_out = np.asarray(b_out, np.float32)

    in_maps = []
    for b in range(B):
        xT = np.ascontiguousarray(x[b].T)  # [D, S]
        for g in range(2):
            heads = range(g * HPG, (g + 1) * HPG)
            # w_qkv rows for head h: [192h, 192h+64) = Q, +64..128 = K, +128..192 = V
            q_rows = np.concatenate([np.arange(3 * HD * h, 3 * HD * h + HD) for h in heads])
            k_rows = q_rows + HD
            v_rows = q_rows + 2 * HD
            wqT = np.ascontiguousarray(w_qkv[q_rows].T)  # [D, E]
            wkT = np.ascontiguousarray(w_qkv[k_rows].T)
            wvT = np.ascontiguousarray(w_qkv[v_rows].T)
            ecols = np.arange(g * E, (g + 1) * E)
            woT = np.ascontiguousarray(w_out[:, ecols].T)  # [E, D]
            bo = b_out[:, None] if g == 0 else np.zeros((D, 1), np.float32)
            in_maps.append({
                "xT": xT,
                "wqT": wqT,
                "wkT": wkT,
                "wvT": wvT,
                "woT": woT,
                "bq": np.ascontiguousarray(b_qkv[q_rows][:, None]),
                "bk": np.ascontiguousarray(b_qkv[k_rows][:, None]),
                "bv": np.ascontiguousarray(b_qkv[v_rows][None, :]),
                "bo": np.ascontiguousarray(bo),
            })
    return in_maps


def run(inputs, trace=False):
    """Run the kernel; returns (full_output, exec_time_ns or None)."""
    nc = _get_nc()
    in_maps = _shard_inputs(**inputs)
    res = run_bass_kernel_spmd(nc, in_maps, core_ids=list(range(8)), trace=trace)
    out = np.empty((B, S, D), np.float32)
    for b in range(B):
        acc = res.results[2 * b]["outT"] + res.results[2 * b + 1]["outT"]
        out[b] = acc.T
    return out, res.exec_time_ns


def kernel(x, w_qkv, b_qkv, w_out, b_out):
    out, _ = run(dict(x=x, w_qkv=w_qkv, b_qkv=b_qkv, w_out=w_out, b_out=b_out))
    return out

